# revision 1
# baseline (speedup 1.0000x reference)
"""Multi-head attention (B=4, S=2048, E=768, H=8, D=96) on 8 Trainium2 cores.

Sharding: core c -> (batch b = c//2, head-group hg = c%2 of 4 heads).
Each core computes Q/K/V projections for its 4 heads over the full sequence
of its batch, full attention for those heads, and a partial output
projection (row-split Wo).  The two cores of a batch produce partial
outputs that are summed on the host during unsharding (tensor-parallel
reduce).

On-chip layout notes:
  - All matmul operands are bf16 (1 cycle/row on PE; fp32 would be 4x).
  - head_dim 96 is zero-padded to 128 (host pads Wq/Wk columns), so every
    matmul has K=128 contraction and 128-column stationary operands (FWL).
  - Attention scores are computed transposed, S^T[k, q] = K^T.T @ Q^T,
    so softmax normalization is a partition reduction; we get the sums for
    free by augmenting V with a ones column (row 96 of the O^T accumulator
    is then sum_k exp(S)).
  - exp runs on the scalar engine straight out of PSUM ([128,1024] over a
    pair of key tiles) with the 1/sqrt(d) scale folded into the
    activation's scale parameter.
  - Per-(head, q-chunk) normalization: sums row -> DRAM -> broadcast-DMA
    to 96 partitions -> fast reciprocal on DVE -> one tensor_tensor mult.
  - The PE stream is software-pipelined: O-matmuls trail S-matmuls by one
    pair, and K/Q projection chunks of head h+1 are interleaved into head
    h's (ACT-bound) attention stream to fill PE slack.
"""

import os
import sys

sys.path.insert(0, "/opt/trn_rl_repo")

import numpy as np
import ml_dtypes

import concourse.bacc as bacc
import concourse.bass as bass
import concourse.tile as tile
from concourse import mybir
from concourse.bass_utils import run_bass_kernel_spmd

BF16 = ml_dtypes.bfloat16

EMB = 768
HEADS = 8
HD = 96          # true head dim
HDP = 128        # padded head dim
SEQ = 2048
B = 4
NCORES = 8
HPC = 4          # heads per core
SCALING = HD ** -0.5
QC = 512         # query chunk per attention inner loop
NQC = SEQ // QC
NKT = SEQ // 128  # 16 key tiles
NPAIR = NKT // 2
NE = EMB // 128   # 6 e_in tiles

_NC_CACHE = {}
LAST_RESULT = None  # BassKernelResults of the most recent run (for test.py)


def _build_nc():
    f32 = mybir.dt.float32
    bf = mybir.dt.bfloat16

    nc = bacc.Bacc(trn_type="TRN2", target_bir_lowering=False, debug=False,
                   num_devices=NCORES)

    xT = nc.dram_tensor("xT", [EMB, SEQ], bf, kind="ExternalInput").ap()
    wqT = nc.dram_tensor("wqT", [EMB, HPC * HDP], bf, kind="ExternalInput").ap()
    wkT = nc.dram_tensor("wkT", [EMB, HPC * HDP], bf, kind="ExternalInput").ap()
    wvT = nc.dram_tensor("wvT", [EMB, HPC * HD], bf, kind="ExternalInput").ap()
    # packed (no head padding): 384 rows = 3 full partition tiles
    woT = nc.dram_tensor("woT", [HPC * HD, EMB], bf, kind="ExternalInput").ap()
    bqp = nc.dram_tensor("bqp", [128, HPC], f32, kind="ExternalInput").ap()
    bkp = nc.dram_tensor("bkp", [128, HPC], f32, kind="ExternalInput").ap()
    outp = nc.dram_tensor("outp", [SEQ, EMB], f32, kind="ExternalOutput").ap()
    sums_dram = nc.dram_tensor("sums_scratch", [HPC * NQC, QC], f32).ap()

    with tile.TileContext(nc) as tc:
        with (
            tc.tile_pool(name="const", bufs=1) as constp,
            tc.tile_pool(name="big", bufs=1) as bigp,
            tc.tile_pool(name="expp", bufs=4) as expp,
            tc.tile_pool(name="rbp", bufs=3) as rbp,
            tc.tile_pool(name="outsb", bufs=3) as outsb,
            tc.tile_pool(name="ps_proj", bufs=2, space="PSUM") as ps_proj,
            tc.tile_pool(name="ps_o", bufs=2, space="PSUM") as ps_o,
            tc.tile_pool(name="ps_pair", bufs=2, space="PSUM") as ps_pair,
        ):
            # ---- loads ----
            # x^T tiles split into two sequence halves so early matmuls
            # don't wait for the full 3MB load.
            # x^T in [128, 512] blocks; emit block 0 + wv first so the first
            # V-projection matmuls start as early as possible.
            xt = [[None] * 4 for _ in range(NE)]

            def load_xt_block(blk):
                for e in range(NE):
                    t = bigp.tile([128, 512], bf, name=f"xt{e}_{blk}")
                    nc.sync.dma_start(
                        out=t, in_=xT[e * 128:(e + 1) * 128,
                                      blk * 512:(blk + 1) * 512])
                    xt[e][blk] = t

            wv_sb = []
            for e in range(NE):
                t = bigp.tile([128, 512], bf, name=f"xt{e}_0")
                nc.sync.dma_start(out=t, in_=xT[e * 128:(e + 1) * 128, 0:512])
                xt[e][0] = t
                t = constp.tile([128, HPC * HD], bf, name=f"wv{e}")
                nc.sync.dma_start(out=t, in_=wvT[e * 128:(e + 1) * 128, :])
                wv_sb.append(t)
            for blk in range(1, 4):
                load_xt_block(blk)

            def xt_cols(e, lo, width):
                blk = lo // 512
                off = lo - blk * 512
                assert off + width <= 512
                return xt[e][blk][:, off:off + width]

            wq_sb, wk_sb, wo_sb = [], [], []
            for e in range(NE):
                t = constp.tile([128, HPC * HDP], bf, name=f"wk{e}")
                nc.sync.dma_start(out=t, in_=wkT[e * 128:(e + 1) * 128, :])
                wk_sb.append(t)
                t = constp.tile([128, HPC * HDP], bf, name=f"wq{e}")
                nc.sync.dma_start(out=t, in_=wqT[e * 128:(e + 1) * 128, :])
                wq_sb.append(t)
            NWO = HPC * HD // 128  # 3 packed Wo row tiles
            for t_ in range(NWO):
                t = constp.tile([128, EMB], bf, name=f"wo{t_}")
                nc.sync.dma_start(out=t, in_=woT[t_ * 128:(t_ + 1) * 128, :])
                wo_sb.append(t)
            bq_sb = constp.tile([128, HPC], f32, name="bq_sb")
            nc.sync.dma_start(out=bq_sb, in_=bqp)
            bk_sb = constp.tile([128, HPC], f32, name="bk_sb")
            nc.sync.dma_start(out=bk_sb, in_=bkp)

            # ---- persistent intermediates ----
            vaug = []
            for kt in range(NKT):
                t = bigp.tile([128, HPC * HDP], bf, name=f"vaug{kt}")
                nc.gpsimd.memset(t, 0.0)
                ones_cols = t.rearrange("p (h c) -> p h c", h=HPC)[:, :, HD:HD + 1]
                nc.gpsimd.memset(ones_cols, 1.0)
                vaug.append(t)
            qT = [bigp.tile([128, SEQ], bf, name=f"qT{h}") for h in range(HPC)]
            kT = [bigp.tile([128, SEQ], bf, name=f"kT{h}") for h in range(HPC)]
            # packed attention output, [384 rows = 3 tiles x 128, seq]; every
            # row is written by the normalization TTs, so no memset needed
            attnT = [bigp.tile([128, SEQ], bf, name=f"attnT{t_}")
                     for t_ in range(NWO)]

            def head_blocks(h):
                """32-row blocks mapping head h's 96 rows into packed attnT
                (all partition starts/spans quadrant-legal)."""
                out = []
                for b_ in range(HD // 32):
                    g = HD * h + 32 * b_
                    out.append((g // 128, g % 128, 32 * b_))
                return out

            f32_ = f32

            # ---- projection emit helpers ----
            def emit_v_chunk(kt):
                psv = ps_proj.tile([128, 512], f32_, tag="ps",
                                   name=f"psv{kt}")
                for e in range(NE):
                    nc.tensor.matmul(psv[:, 0:HPC * HD],
                                     lhsT=xt_cols(e, kt * 128, 128),
                                     rhs=wv_sb[e],
                                     start=(e == 0), stop=(e == NE - 1))
                for hh in range(HPC):
                    nc.vector.tensor_copy(
                        vaug[kt][:, hh * HDP:hh * HDP + HD],
                        psv[:, hh * HD:(hh + 1) * HD])

            def emit_kq_chunk(h, n, which):
                nsl = slice(n * 512, (n + 1) * 512)
                w_sb, dst, b_sb = ((wk_sb, kT, bk_sb) if which == "k"
                                   else (wq_sb, qT, bq_sb))
                ps = ps_proj.tile([128, 512], f32_, tag="ps",
                                  name=f"ps{which}{h}_{n}")
                for e in range(NE):
                    nc.tensor.matmul(ps,
                                     lhsT=w_sb[e][:, h * HDP:(h + 1) * HDP],
                                     rhs=xt_cols(e, n * 512, 512),
                                     start=(e == 0), stop=(e == NE - 1))
                nc.vector.tensor_scalar_add(dst[h][:, nsl], ps,
                                            b_sb[:, h:h + 1])

            def kq_chunks(h):
                for n in range(4):
                    yield ("k", h, n)
                for n in range(4):
                    yield ("q", h, n)

            # ---- output projection chunk (one 128-row q tile) ----
            # Split across two 1-bank psums so it can borrow ps_proj slots;
            # PSUM->SBUF copies go on DVE (ACT is busy with exp here).
            def emit_out_chunk(qm):
                qsl = slice(qm * 128, (qm + 1) * 128)
                psA = ps_proj.tile([128, 512], f32_, tag="ps",
                                   name=f"poA{qm}")
                psB = ps_proj.tile([128, 512], f32_, tag="ps",
                                   name=f"poB{qm}")
                for t in range(NWO):
                    nc.tensor.matmul(psA,
                                     lhsT=attnT[t][:, qsl],
                                     rhs=wo_sb[t][:, 0:512],
                                     start=(t == 0), stop=(t == NWO - 1))
                for t in range(NWO):
                    nc.tensor.matmul(psB[:, 0:256],
                                     lhsT=attnT[t][:, qsl],
                                     rhs=wo_sb[t][:, 512:768],
                                     start=(t == 0), stop=(t == NWO - 1))
                out_sb = outsb.tile([128, EMB], f32_, tag="osb",
                                    name=f"osb{qm}")
                nc.vector.tensor_copy(out_sb[:, 0:512], psA)
                nc.vector.tensor_copy(out_sb[:, 512:768], psB[:, 0:256])
                nc.sync.dma_start(out=outp[qm * 128:(qm + 1) * 128, :],
                                  in_=out_sb)

            # ---- attention emit (with interleaved PE filler work) ----
            def emit_attention(h, thunks_for_qc):
                """thunks_for_qc(qc) -> list of emit callables injected into
                the PE stream spread across this q-chunk's pairs."""
                hsl = slice(h * HDP, (h + 1) * HDP)
                for qc in range(NQC):
                    thunks = list(thunks_for_qc(qc))
                    step = NPAIR // max(len(thunks), 1)
                    inject_at = {(1 + i * step) % NPAIR: t
                                 for i, t in enumerate(thunks)}
                    qsl = slice(qc * QC, (qc + 1) * QC)
                    idx = h * NQC + qc
                    pso = ps_o.tile([128, QC], f32_, tag="pso",
                                    name=f"pso{idx}")
                    eps = []

                    def emit_ss(p):
                        pss = ps_pair.tile([128, 1024], f32_, tag="pss",
                                           name=f"pss{idx}_{p}")
                        for j in range(2):
                            nc.tensor.matmul(
                                pss[:, j * 512:(j + 1) * 512],
                                lhsT=kT[h][:, (2 * p + j) * 128:
                                           (2 * p + j + 1) * 128],
                                rhs=qT[h][:, qsl],
                                start=True, stop=True)
                        ep = expp.tile([128, 1024], bf, tag="exp",
                                       name=f"exp{idx}_{p}")
                        nc.scalar.activation(ep, pss,
                                             mybir.ActivationFunctionType.Exp,
                                             scale=SCALING)
                        eps.append(ep)

                    def emit_o(p):
                        for j in range(2):
                            kt = 2 * p + j
                            nc.tensor.matmul(
                                pso,
                                lhsT=vaug[kt][:, hsl],
                                rhs=eps[p][:, j * 512:(j + 1) * 512],
                                start=(kt == 0), stop=(kt == NKT - 1))

                    for p in range(NPAIR):
                        emit_ss(p)
                        if p in inject_at:
                            inject_at[p]()
                        if p >= 1:
                            emit_o(p - 1)
                    emit_o(NPAIR - 1)

                    sums_sb = rbp.tile([1, QC], f32_, tag="sums",
                                       name=f"sums{idx}")
                    nc.vector.tensor_copy(sums_sb, pso[HD:HD + 1, :])
                    nc.sync.dma_start(out=sums_dram[idx:idx + 1, :],
                                      in_=sums_sb)
                    rb = rbp.tile([HD, QC], f32_, tag="rb", name=f"rb{idx}")
                    nc.sync.dma_start(
                        out=rb,
                        in_=sums_dram[idx:idx + 1, :].to_broadcast([HD, QC]))
                    rb2 = rbp.tile([HD, QC], f32_, tag="rb2",
                                   name=f"rb2{idx}")
                    nc.vector.reciprocal_approx_fast(out=rb2, in_=rb)
                    for t_, off, src in head_blocks(h):
                        nc.vector.tensor_mul(
                            out=attnT[t_][off:off + 32, qsl],
                            in0=pso[src:src + 32, :],
                            in1=rb2[src:src + 32, :])

            # ---- emission schedule ----
            for kt in range(NKT):
                emit_v_chunk(kt)
            for which, hh, n in kq_chunks(0):
                emit_kq_chunk(hh, n, which)

            def kq_thunks(hnext):
                def f(qc):
                    # 2 chunks per q-chunk: 8 chunks over 4 qcs
                    items = list(kq_chunks(hnext))[2 * qc:2 * qc + 2]
                    return [lambda it=it: emit_kq_chunk(it[1], it[2], it[0])
                            for it in items]
                return f

            def out_thunks(qc):
                # during h3's q-chunk qc, emit out-proj rows of q-chunk qc-1
                if qc == 0:
                    return []
                return [lambda qm=qm: emit_out_chunk(qm)
                        for qm in range(4 * (qc - 1), 4 * qc)]

            for h in range(HPC - 1):
                emit_attention(h, kq_thunks(h + 1))
            emit_attention(HPC - 1, out_thunks)
            for qm in range(4 * (NQC - 1), 4 * NQC):
                emit_out_chunk(qm)

    nc.compile()
    return nc


def _get_nc():
    if "nc" not in _NC_CACHE:
        _NC_CACHE["nc"] = _build_nc()
    return _NC_CACHE["nc"]


def _pad_headsT(w_rows):
    """[384, 768] head rows -> zero-pad head dim 96->128 -> transpose -> [768, 512]."""
    p = np.zeros((HPC * HDP, EMB), np.float32)
    p.reshape(HPC, HDP, EMB)[:, :HD] = w_rows.reshape(HPC, HD, EMB)
    return np.ascontiguousarray(p.T).astype(BF16)


def _pad_bias(b_rows):
    """[384] head bias -> [128, HPC] padded/transposed for per-partition add."""
    p = np.zeros((HPC, HDP), np.float32)
    p[:, :HD] = b_rows.reshape(HPC, HD)
    return np.ascontiguousarray(p.T)


def kernel(x, Wq, bq, Wk, bk, Wv, bv, Wo, bo):
    x = np.asarray(x, np.float32)
    Wq, bq = np.asarray(Wq, np.float32), np.asarray(bq, np.float32)
    Wk, bk = np.asarray(Wk, np.float32), np.asarray(bk, np.float32)
    Wv, bv = np.asarray(Wv, np.float32), np.asarray(bv, np.float32)
    Wo, bo = np.asarray(Wo, np.float32), np.asarray(bo, np.float32)

    nc = _get_nc()

    in_maps = []
    for c in range(NCORES):
        b, hg = divmod(c, 2)
        hs = slice(hg * HPC * HD, (hg + 1) * HPC * HD)
        in_maps.append({
            "xT": np.ascontiguousarray(x[b].T).astype(BF16),
            "wqT": _pad_headsT(Wq[hs]),
            "wkT": _pad_headsT(Wk[hs]),
            "wvT": np.ascontiguousarray(Wv[hs].T).astype(BF16),
            "woT": np.ascontiguousarray(Wo[:, hs].T).astype(BF16),
            "bqp": _pad_bias(bq[hs]),
            "bkp": _pad_bias(bk[hs]),
        })

    global LAST_RESULT
    trace = bool(int(os.environ.get("KERNEL_TRACE", "0")))
    tmpdir = os.environ.get("KERNEL_TRACE_DIR") or None
    res = run_bass_kernel_spmd(nc, in_maps, list(range(NCORES)), trace=trace,
                               tmpdir=tmpdir)
    LAST_RESULT = res

    out = np.empty((B, SEQ, EMB), np.float32)
    for b in range(B):
        out[b] = res.results[2 * b]["outp"] + res.results[2 * b + 1]["outp"]
    # bv enters each head's output additively (sum of softmax weights is 1),
    # and bo is a plain add: both fold into one constant vector.
    out += Wo @ bv + bo
    return out

